# revision 31
# baseline (speedup 1.0000x reference)
"""Trainium2 Bass kernel for nn_MultiHeadAttentionBlock_49967649521921.

Reference computation (per batch b, x viewed as [C=512, N=1024]):
    q = Wq @ x ; k = Wk @ x ; v = Wv @ x          (1x1 convs, biases zero)
    per head h (8 heads, hd=64):
      scores[d,e] = sum_n q_h[d,n] k_h[e,n] / 8
      attn = softmax(scores, axis=e)
      out_h[d,n]  = sum_e attn[d,e] v_h[e,n]
    y[c',s'] with c' = h*64 + n//16, s' = (n%16)*64 + d
    final = Wo @ y    -> reshape [512, 32, 32]

v8 design — Gram-matrix restructure with a transposed softmax that never
touches the PE transpose path:
    G   = X X^T                 (per batch, [512,512])
    T   = G Wk^T                ([512,512])
    S^T  per head-pair: lhs/rhs swapped so scores come out [e,d]
    exp  (Act) writes the unnormalized exp(S^T) straight into the
         block-diagonal slots of at_bd
    rowsums over e via a ones-vector PE matmul; DVE reciprocal;
         gpsimd partition-broadcast -> rcf[128, (h,d)]
    A'T_unnorm = Wv-chunks^T @ at_bd; the psum->SBUF copy is a DVE
         tensor_tensor multiply by rcf (this IS the softmax normalize —
         scale by 1/rowsum folds into A'T's columns)
    out^T[m,(h,d)] = X^T A'T    (m-order n=16a+r -> m=64r+a makes the
                                 reference transpose(2,3) a strided copy)
    final = Wo^T-chunks^T @ y

DMA facts this layout exploits: per-queue cost is ~30ns per DESCRIPTOR
(one per partition-run), so any [128,..] load is ~4us; latency-critical
tiles are split by partition range across the two HWDGE queues, and
outputs are folded to 64 partitions (64 descriptors). SWDGE (gpsimd)
has ~4us start latency and carries only late-use loads. The Act-engine
exp-table load is moved after the early DMA triggers post-compile (it
otherwise blocks the scalar HWDGE ring for ~2us).
"""

import os
import sys

import numpy as np

for _p in ("/opt/trn_rl_repo",):
    if _p not in sys.path and os.path.isdir(_p):
        sys.path.insert(0, _p)

from contextlib import ExitStack

import concourse.bass as bass
import concourse.tile as tile
from concourse import bacc
from concourse import mybir
from concourse.bass_utils import run_bass_kernel_spmd

F32 = mybir.dt.float32
BF16 = mybir.dt.bfloat16
AF = mybir.ActivationFunctionType

N_CORES = 8
B_PER_CORE = 2
C = 512
N = 1024
NH = 8
HD = 64


def _split_excess_dma_waits(nc):
    """walrus' static-DMA (PSEUDO_DMA_DIRECT2D) encoding accepts a single
    sync-wait; Bacc's generate_event_semaphores only splits waits on compute
    instructions. Move excess DMA waits onto preceding EventSemaphore
    carriers (2 waits each) on the same engine queue."""
    for f in nc.m.functions:
        for blk in f.blocks:
            changed = False
            new_insts = []
            for inst in blk.instructions:
                si = inst.sync_info
                waits = list(si.on_wait) if si is not None and si.on_wait else []
                if inst.opcode == "DMACopy" and len(waits) > 1:
                    keep, excess = waits[:1], waits[1:]
                    k = 0
                    while excess:
                        chunk, excess = excess[:2], excess[2:]
                        ev = mybir.InstEventSemaphore(
                            name=f"{inst.name}-evw{k}",
                            opcode="EventSemaphore",
                            engine=inst.engine,
                            sync_info=mybir.SyncInfo(on_wait=chunk, on_update=[]),
                        )
                        new_insts.append(ev)
                        k += 1
                    inst.sync_info = mybir.SyncInfo(
                        on_wait=keep, on_update=list(si.on_update or [])
                    )
                    changed = True
                new_insts.append(inst)
            if changed:
                blk.instructions = new_insts


def _postpone_act_table_load(nc):
    """insert_act_table_loads puts LoadActFuncSet at the top of the block,
    where it stalls the Activation engine's HWDGE ring for ~2us right when
    the latency-critical first loads should be dispatching. Move it after
    the last early Activation-engine DMACopy (still far before the first
    Activation instruction)."""
    for f in nc.m.functions:
        for blk in f.blocks:
            insts = blk.instructions
            tbl_idx = None
            last_dma_idx = None
            first_act_idx = None
            for idx, inst in enumerate(insts):
                if inst.engine != mybir.EngineType.Activation:
                    continue
                if inst.opcode == "LoadActFuncSet":
                    tbl_idx = idx
                elif inst.opcode == "DMACopy" and first_act_idx is None:
                    last_dma_idx = idx
                elif inst.opcode == "Activation" and first_act_idx is None:
                    first_act_idx = idx
            if tbl_idx is None or last_dma_idx is None:
                continue
            if tbl_idx < last_dma_idx and (
                first_act_idx is None or last_dma_idx < first_act_idx
            ):
                inst = insts.pop(tbl_idx)
                insts.insert(last_dma_idx, inst)


def build_program():
    nc = bacc.Bacc("TRN2", target_bir_lowering=False, debug=False)

    xt_d = nc.dram_tensor("xt", [B_PER_CORE, 128, 8, C], BF16, kind="ExternalInput").ap()
    xc_d = nc.dram_tensor("xc", [B_PER_CORE, 128, 4, N], BF16, kind="ExternalInput").ap()
    wkt_d = nc.dram_tensor("wkt", [128, 4, C], BF16, kind="ExternalInput").ap()
    wqt_d = nc.dram_tensor("wqt", [128, 4, C], BF16, kind="ExternalInput").ap()
    wv_d = nc.dram_tensor("wv", [128, 4, C], BF16, kind="ExternalInput").ap()
    wot_d = nc.dram_tensor("wot", [128, 4, C], BF16, kind="ExternalInput").ap()
    out_d = nc.dram_tensor(
        "out", [B_PER_CORE, 2, 64, 4, 2, 512], BF16, kind="ExternalOutput"
    ).ap()

    with tile.TileContext(nc) as tc, ExitStack() as ctx:
        wp = ctx.enter_context(tc.tile_pool(name="w", bufs=1))
        xtp = ctx.enter_context(tc.tile_pool(name="xt", bufs=2))
        xcp = ctx.enter_context(tc.tile_pool(name="xc", bufs=2))
        gp = ctx.enter_context(tc.tile_pool(name="g", bufs=2))
        tp = ctx.enter_context(tc.tile_pool(name="t", bufs=2))
        smp = ctx.enter_context(tc.tile_pool(name="sm", bufs=2))
        ap_ = ctx.enter_context(tc.tile_pool(name="apt", bufs=2))
        yp = ctx.enter_context(tc.tile_pool(name="y", bufs=2))
        ogp = ctx.enter_context(tc.tile_pool(name="og", bufs=4))

        # PSUM: 8 banks.  acc (4) rotates through G/T/outT/final chunks;
        # s (2) per-head-pair score tiles; a (2) rowsums + A'T chunks.
        ps_acc = ctx.enter_context(tc.tile_pool(name="psacc", bufs=4, space="PSUM"))
        ps_s = ctx.enter_context(tc.tile_pool(name="pss", bufs=2, space="PSUM"))
        ps_a = ctx.enter_context(tc.tile_pool(name="psa", bufs=2, space="PSUM"))

        st = [{} for _ in range(B_PER_CORE)]

        def s_gram(b):
            """G[i*128:(i+1)*128, :] = sum_mc xt[mc]-slice^T @ xt[mc].
            All four g copies on DVE so consumers wait one proc."""
            xt_sb = st[b]["xt"]
            g_sb = gp.tile([128, 4, C], BF16, tag="g", name=f"g{b}")
            st[b]["g"] = g_sb
            pts = [
                ps_acc.tile([128, C], F32, tag="acc", name=f"pg{b}_{i}")
                for i in range(4)
            ]
            for mc in range(8):
                for i in range(4):
                    nc.tensor.matmul(
                        pts[i][:, :],
                        xt_sb[:, mc, i * 128 : (i + 1) * 128],
                        xt_sb[:, mc, :],
                        start=(mc == 0), stop=(mc == 7),
                    )
            for i in range(4):
                if i % 2 == 0:
                    nc.vector.tensor_copy(g_sb[:, i, :], pts[i][:, :])
                else:
                    nc.scalar.copy(g_sb[:, i, :], pts[i][:, :])

        def s_t(b):
            """T = G @ WkT; all four t copies on Act."""
            g_sb = st[b]["g"]
            wkt_sb = st[b]["wkt"]
            t_sb = tp.tile([128, 4, C], BF16, tag="t", name=f"t{b}")
            st[b]["t"] = t_sb
            pts = [
                ps_acc.tile([128, C], F32, tag="acc", name=f"pt{b}_{i}")
                for i in range(4)
            ]
            for j in range(4):
                for i in range(4):
                    nc.tensor.matmul(
                        pts[i][:, :],
                        g_sb[:, j, i * 128 : (i + 1) * 128],
                        wkt_sb[:, j, :],
                        start=(j == 0), stop=(j == 3),
                    )
            for i in range(4):
                if i % 2 == 0:
                    nc.vector.tensor_copy(t_sb[:, i, :], pts[i][:, :])
                else:
                    nc.scalar.copy(t_sb[:, i, :], pts[i][:, :])

        def s_scores(b):
            """scores per head-pair hp in its own PSUM tile; softmax over e
            (Act exp with free-axis accum, DVE recip, Act copy-scale); the
            normalized attn rows land in the block-diagonal slots of
            abd_pre, which the XBAR dma transposes into at_bd = attn^T."""
            wqt_sb = st[b]["wqt"]
            t_sb = st[b]["t"]
            es = smp.tile([128, 4, HD], BF16, tag="es", name=f"es{b}")
            rs = smp.tile([128, 4, 1], F32, tag="rs", name=f"rs{b}")
            rcp = smp.tile([128, 4, 1], F32, tag="rcp", name=f"rcp{b}")
            abd_pre = smp.tile([128, 4, 128], BF16, tag="abdp", name=f"abd_pre{b}")
            at_bd = smp.tile([128, 4, 128], BF16, tag="atbd", name=f"at_bd{b}")
            nc.vector.memset(abd_pre[:, :, :], 0.0)
            st[b]["at_bd"] = at_bd
            for hp in range(4):
                ps1 = ps_s.tile([128, 128], F32, tag="s1", name=f"ps1_{b}_{hp}")
                for j in range(4):
                    nc.tensor.matmul(
                        ps1[:, :],
                        wqt_sb[:, j, hp * 128 : (hp + 1) * 128],
                        t_sb[:, j, hp * 128 : (hp + 1) * 128],
                        start=(j == 0), stop=(j == 3),
                    )
                for hh in range(2):
                    psl = slice(hh * 64, hh * 64 + 64)
                    nc.scalar.activation(
                        es[psl, hp, :], ps1[psl, psl],
                        AF.Exp, scale=0.125,
                        accum_out=rs[psl, hp, :],
                    )
                nc.vector.reciprocal(rcp[:, hp, :], rs[:, hp, :])
                for hh in range(2):
                    psl = slice(hh * 64, hh * 64 + 64)
                    nc.scalar.activation(
                        abd_pre[psl, hp, psl], es[psl, hp, :],
                        AF.Copy, scale=rcp[psl, hp, :],
                    )
            # blockdiag(attn)^T == blockdiag(attn^T): transpose each pair
            # tile SBUF->SBUF on the DMA XBAR (off PE/PSUM entirely)
            for hp in range(4):
                eng = nc.sync if hp % 2 == 0 else nc.scalar
                eng.dma_start(
                    at_bd[:, hp, :], abd_pre[:, hp, :], transpose=True
                )

        def s_apt(b):
            """A\'T[i-chunk, (h,d)] = wv-chunks^T @ at_bd (all copies DVE)."""
            wv_sb = st[b]["wv"]
            at_bd = st[b]["at_bd"]
            apt_sb = ap_.tile([128, 4, C], BF16, tag="apt", name=f"apt{b}")
            st[b]["apt"] = apt_sb
            for i in range(4):
                pa = ps_a.tile([128, C], F32, tag="a", name=f"pa{b}_{i}")
                for hp in range(4):
                    nc.tensor.matmul(
                        pa[:, hp * 128 : (hp + 1) * 128],
                        wv_sb[:, hp, i * 128 : (i + 1) * 128],
                        at_bd[:, hp, :],
                        start=True, stop=True,
                    )
                if i % 2 == 0:
                    nc.vector.tensor_copy(apt_sb[:, i, :], pa[:, :])
                else:
                    nc.scalar.copy(apt_sb[:, i, :], pa[:, :])

        def s_outt(b):
            """outT[mc-chunk, (h,d)] = sum_j xc[j, mc-slice]^T @ A'T[j].
            The psum->y copies (all Act) realize the transpose(2,3)
            scramble: y[(h%2)*64+a, h//2, (2mc+rr)*64+d] = outT[rr*64+a, (h,d)]."""
            xc_sb = st[b]["xc"]
            apt_sb = st[b]["apt"]
            y_sb = yp.tile([128, 4, N], BF16, tag="y", name=f"y{b}")
            st[b]["y"] = y_sb
            for mc in range(8):
                po = ps_acc.tile([128, 4, 128], F32, tag="acc", name=f"po{b}_{mc}")
                for j in range(4):
                    nc.tensor.matmul(
                        po[:, :, :],
                        xc_sb[:, j, mc * 128 : (mc + 1) * 128],
                        apt_sb[:, j, :],
                        start=(j == 0), stop=(j == 3),
                    )
                k = 0
                for rr in range(2):
                    for par in range(2):
                        dst = y_sb[
                            par * 64 : par * 64 + 64,
                            :,
                            (2 * mc + rr) * 64 : (2 * mc + rr) * 64 + 64,
                        ]
                        src_ = po[
                            rr * 64 : rr * 64 + 64, :, par * 64 : par * 64 + 64
                        ]
                        if (mc + k) % 4 == 1:
                            nc.scalar.copy(dst, src_)
                        else:
                            nc.vector.tensor_copy(dst, src_)
                        k += 1

        def s_final(b):
            """final[oc-chunk, :] = sum_j wot[j, oc-slice]^T @ y[j, :].
            j-outer / sh-inner shares each LDWEIGHTS across two matmuls.
            og is folded to 64 partitions: og64[p2, oc, k, s] =
            final[oc*128 + k*64 + p2, sh*512+s]; all og copies on DVE."""
            wot_sb = st[b]["wot"]
            y_sb = st[b]["y"]
            ogs = [
                ogp.tile([64, 4, 2, 512], BF16, tag="og", name=f"og{b}_{sh}")
                for sh in range(2)
            ]
            for oc in range(4):
                pf = [
                    ps_acc.tile([128, C], F32, tag="acc", name=f"pf{b}_{oc}_{sh}")
                    for sh in range(2)
                ]
                for j in range(4):
                    for sh in range(2):
                        nc.tensor.matmul(
                            pf[sh][:, :],
                            wot_sb[:, j, oc * 128 : (oc + 1) * 128],
                            y_sb[:, j, sh * 512 : (sh + 1) * 512],
                            start=(j == 0), stop=(j == 3),
                        )
                for sh in range(2):
                    for k in range(2):
                        if (oc + sh + k) % 2 == 0:
                            nc.vector.tensor_copy(
                                ogs[sh][:, oc, k, :],
                                pf[sh][k * 64 : (k + 1) * 64, :],
                            )
                        else:
                            nc.scalar.copy(
                                ogs[sh][:, oc, k, :],
                                pf[sh][k * 64 : (k + 1) * 64, :],
                            )
                if b == 1 and oc == 1:
                    nc.sync.dma_start(
                        out_d[1, 0, :, 0:2, :, :], ogs[0][:, 0:2, :, :]
                    )
                    nc.scalar.dma_start(
                        out_d[1, 1, :, 0:2, :, :], ogs[1][:, 0:2, :, :]
                    )
            if b == 0:
                nc.sync.dma_start(out_d[0, 0, :, :, :, :], ogs[0][:, :, :, :])
                nc.scalar.dma_start(out_d[0, 1, :, :, :, :], ogs[1][:, :, :, :])
            else:
                nc.sync.dma_start(out_d[1, 0, :, 2:4, :, :], ogs[0][:, 2:4, :, :])
                nc.scalar.dma_start(out_d[1, 1, :, 2:4, :, :], ogs[1][:, 2:4, :, :])

        # ---- loads ----
        xt0 = xtp.tile([128, 8, C], BF16, tag="xt", name="xt_sb0")
        xt1 = xtp.tile([128, 8, C], BF16, tag="xt", name="xt_sb1")
        xc0 = xcp.tile([128, 4, N], BF16, tag="xc", name="xc_sb0")
        xc1 = xcp.tile([128, 4, N], BF16, tag="xc", name="xc_sb1")
        st[0]["xt"], st[1]["xt"] = xt0, xt1
        st[0]["xc"], st[1]["xc"] = xc0, xc1

        nc.sync.dma_start(xt0[0:64, 0:4, :], xt_d[0, 0:64, 0:4, :])
        nc.scalar.dma_start(xt0[64:128, 0:4, :], xt_d[0, 64:128, 0:4, :])
        nc.sync.dma_start(xt0[0:64, 4:8, :], xt_d[0, 0:64, 4:8, :])
        nc.scalar.dma_start(xt0[64:128, 4:8, :], xt_d[0, 64:128, 4:8, :])

        w_sb = {}
        for wname in ("wkt", "wqt", "wv", "wot"):
            t = wp.tile([128, 4, C], BF16, tag=wname, name=f"w_{wname}")
            w_sb[wname] = t
            for b in range(B_PER_CORE):
                st[b][wname] = t
        nc.sync.dma_start(xt1[:, :, :], xt_d[1, :, :, :])
        nc.scalar.dma_start(w_sb["wkt"][:, :, :], wkt_d[:, :, :])
        nc.scalar.dma_start(w_sb["wqt"][:, :, :], wqt_d[:, :, :])
        nc.gpsimd.dma_start(xc0[:, :, :], xc_d[0, :, :, :])
        nc.gpsimd.dma_start(w_sb["wv"][:, :, :], wv_d[:, :, :])
        nc.gpsimd.dma_start(w_sb["wot"][:, :, :], wot_d[:, :, :])
        nc.gpsimd.dma_start(xc1[:, :, :], xc_d[1, :, :, :])

        # ---- schedule (a hint: the tile scheduler statically reorders
        # per-engine streams with its cost model) ----
        s_gram(0)
        s_t(0)
        s_scores(0)
        s_gram(1)
        s_t(1)
        s_scores(1)
        s_apt(0)
        s_apt(1)
        s_outt(0)
        s_outt(1)
        s_final(0)
        s_final(1)

    nc.compile()
    _split_excess_dma_waits(nc)
    if os.environ.get("K_MOVE_ACT_TABLE", "1") == "1":
        _postpone_act_table_load(nc)
    return nc


_PROGRAM = None


def _get_program():
    global _PROGRAM
    if _PROGRAM is None:
        _PROGRAM = build_program()
    return _PROGRAM


def make_in_maps(x, Wq, Wk, Wv, Wo):
    import ml_dtypes

    bf = ml_dtypes.bfloat16
    # permute spatial axis n = 16a + r -> m = 64r + a
    xm = (
        x.reshape(16, C, 64, 16)
        .transpose(0, 1, 3, 2)
        .reshape(16, C, N)
    )
    # xc: [b, 128, cc, m]  (X, channel-partition)
    xc = np.ascontiguousarray(
        xm.reshape(16, 4, 128, N).transpose(0, 2, 1, 3).astype(bf)
    )
    # xt: [b, 128, mc, c]  (X^T, m-partition)
    xt = np.ascontiguousarray(
        xm.transpose(0, 2, 1).reshape(16, 8, 128, C).transpose(0, 2, 1, 3).astype(bf)
    )

    def _wt(W):
        return np.ascontiguousarray(
            W.T.reshape(4, 128, C).transpose(1, 0, 2).astype(bf)
        )

    wkt, wqt, wot = _wt(Wk), _wt(Wq), _wt(Wo)
    # wv: [128 (hh,e), hp, c]: row (2*hp+hh)*64+e of Wv
    wv = np.ascontiguousarray(
        Wv.reshape(4, 2, HD, C).transpose(1, 2, 0, 3).reshape(128, 4, C).astype(bf)
    )
    in_maps = []
    for c in range(N_CORES):
        bsl = slice(c * B_PER_CORE, (c + 1) * B_PER_CORE)
        in_maps.append(
            {
                "xt": np.ascontiguousarray(xt[bsl]),
                "xc": np.ascontiguousarray(xc[bsl]),
                "wkt": wkt,
                "wqt": wqt,
                "wv": wv,
                "wot": wot,
            }
        )
    return in_maps


def kernel(x, Wq, bq, Wk, bk, Wv, bv, Wo, bo, _trace=False):
    # biases are zeros by construction in this problem (spec fill="zeros");
    # they are not applied on-device.
    nc = _get_program()
    in_maps = make_in_maps(x, Wq, Wk, Wv, Wo)
    res = run_bass_kernel_spmd(nc, in_maps, list(range(N_CORES)), trace=_trace)
    outs = [
        np.asarray(res.results[c]["out"]).astype(np.float32) for c in range(N_CORES)
    ]
    # out [b, sh, p2, oc, k, s] -> F[b, oc*128 + k*64 + p2, sh*512 + s];
    # the spatial index is already the true flat index (no inverse perm)
    full = (
        np.concatenate(outs, axis=0)
        .transpose(0, 3, 4, 2, 1, 5)
        .reshape(16, C, N)
        .reshape(16, C, 32, 32)
    )
    if _trace:
        return full, res
    return full


# revision 32
# speedup vs baseline: 1.1288x; 1.1288x over previous
"""Trainium2 Bass kernel for nn_MultiHeadAttentionBlock_49967649521921.

Reference computation (per batch b, x viewed as [C=512, N=1024]):
    q = Wq @ x ; k = Wk @ x ; v = Wv @ x          (1x1 convs, biases zero)
    per head h (8 heads, hd=64):
      scores[d,e] = sum_n q_h[d,n] k_h[e,n] / 8
      attn = softmax(scores, axis=e)
      out_h[d,n]  = sum_e attn[d,e] v_h[e,n]
    y[c',s'] with c' = h*64 + n//16, s' = (n%16)*64 + d
    final = Wo @ y    -> reshape [512, 32, 32]

v8 design — Gram-matrix restructure with a transposed softmax that never
touches the PE transpose path:
    G   = X X^T                 (per batch, [512,512])
    T   = G Wk^T                ([512,512])
    S^T  per head-pair: lhs/rhs swapped so scores come out [e,d]
    exp  (Act) writes the unnormalized exp(S^T) straight into the
         block-diagonal slots of at_bd
    rowsums over e via a ones-vector PE matmul; DVE reciprocal;
         gpsimd partition-broadcast -> rcf[128, (h,d)]
    A'T_unnorm = Wv-chunks^T @ at_bd; the psum->SBUF copy is a DVE
         tensor_tensor multiply by rcf (this IS the softmax normalize —
         scale by 1/rowsum folds into A'T's columns)
    out^T[m,(h,d)] = X^T A'T    (m-order n=16a+r -> m=64r+a makes the
                                 reference transpose(2,3) a strided copy)
    final = Wo^T-chunks^T @ y

DMA facts this layout exploits: per-queue cost is ~30ns per DESCRIPTOR
(one per partition-run), so any [128,..] load is ~4us; latency-critical
tiles are split by partition range across the two HWDGE queues, and
outputs are folded to 64 partitions (64 descriptors). SWDGE (gpsimd)
has ~4us start latency and carries only late-use loads. The Act-engine
exp-table load is moved after the early DMA triggers post-compile (it
otherwise blocks the scalar HWDGE ring for ~2us).
"""

import os
import sys

import numpy as np

for _p in ("/opt/trn_rl_repo",):
    if _p not in sys.path and os.path.isdir(_p):
        sys.path.insert(0, _p)

from contextlib import ExitStack

import concourse.bass as bass
import concourse.tile as tile
from concourse import bacc
from concourse import mybir
from concourse.bass_utils import run_bass_kernel_spmd

F32 = mybir.dt.float32
BF16 = mybir.dt.bfloat16
AF = mybir.ActivationFunctionType

N_CORES = 8
B_PER_CORE = 2
C = 512
N = 1024
NH = 8
HD = 64


def _split_excess_dma_waits(nc):
    """walrus' static-DMA (PSEUDO_DMA_DIRECT2D) encoding accepts a single
    sync-wait; Bacc's generate_event_semaphores only splits waits on compute
    instructions. Move excess DMA waits onto preceding EventSemaphore
    carriers (2 waits each) on the same engine queue."""
    for f in nc.m.functions:
        for blk in f.blocks:
            changed = False
            new_insts = []
            for inst in blk.instructions:
                si = inst.sync_info
                waits = list(si.on_wait) if si is not None and si.on_wait else []
                if inst.opcode == "DMACopy" and len(waits) > 1:
                    keep, excess = waits[:1], waits[1:]
                    k = 0
                    while excess:
                        chunk, excess = excess[:2], excess[2:]
                        ev = mybir.InstEventSemaphore(
                            name=f"{inst.name}-evw{k}",
                            opcode="EventSemaphore",
                            engine=inst.engine,
                            sync_info=mybir.SyncInfo(on_wait=chunk, on_update=[]),
                        )
                        new_insts.append(ev)
                        k += 1
                    inst.sync_info = mybir.SyncInfo(
                        on_wait=keep, on_update=list(si.on_update or [])
                    )
                    changed = True
                new_insts.append(inst)
            if changed:
                blk.instructions = new_insts


def _postpone_act_table_load(nc):
    """insert_act_table_loads puts LoadActFuncSet at the top of the block,
    where it stalls the Activation engine's HWDGE ring for ~2us right when
    the latency-critical first loads should be dispatching. Move it after
    the last early Activation-engine DMACopy (still far before the first
    Activation instruction)."""
    for f in nc.m.functions:
        for blk in f.blocks:
            insts = blk.instructions
            tbl_idx = None
            last_dma_idx = None
            first_act_idx = None
            for idx, inst in enumerate(insts):
                if inst.engine != mybir.EngineType.Activation:
                    continue
                if inst.opcode == "LoadActFuncSet":
                    tbl_idx = idx
                elif inst.opcode == "DMACopy" and first_act_idx is None:
                    last_dma_idx = idx
                elif inst.opcode == "Activation" and first_act_idx is None:
                    first_act_idx = idx
            if tbl_idx is None or last_dma_idx is None:
                continue
            if tbl_idx < last_dma_idx and (
                first_act_idx is None or last_dma_idx < first_act_idx
            ):
                inst = insts.pop(tbl_idx)
                insts.insert(last_dma_idx, inst)


def build_program():
    nc = bacc.Bacc("TRN2", target_bir_lowering=False, debug=False)

    xt_d = nc.dram_tensor("xt", [B_PER_CORE, 128, 8, C], BF16, kind="ExternalInput").ap()
    xc_d = nc.dram_tensor("xc", [B_PER_CORE, 128, 4, N], BF16, kind="ExternalInput").ap()
    wkt_d = nc.dram_tensor("wkt", [128, 4, C], BF16, kind="ExternalInput").ap()
    wqt_d = nc.dram_tensor("wqt", [128, 4, C], BF16, kind="ExternalInput").ap()
    wv_d = nc.dram_tensor("wv", [128, 4, C], BF16, kind="ExternalInput").ap()
    wot_d = nc.dram_tensor("wot", [128, 4, C], BF16, kind="ExternalInput").ap()
    out_d = nc.dram_tensor(
        "out", [B_PER_CORE, 2, 64, 4, 2, 512], BF16, kind="ExternalOutput"
    ).ap()

    with tile.TileContext(nc) as tc, ExitStack() as ctx:
        wp = ctx.enter_context(tc.tile_pool(name="w", bufs=1))
        xtp = ctx.enter_context(tc.tile_pool(name="xt", bufs=2))
        xcp = ctx.enter_context(tc.tile_pool(name="xc", bufs=2))
        gp = ctx.enter_context(tc.tile_pool(name="g", bufs=2))
        tp = ctx.enter_context(tc.tile_pool(name="t", bufs=2))
        smp = ctx.enter_context(tc.tile_pool(name="sm", bufs=2))
        ap_ = ctx.enter_context(tc.tile_pool(name="apt", bufs=2))
        yp = ctx.enter_context(tc.tile_pool(name="y", bufs=2))
        ogp = ctx.enter_context(tc.tile_pool(name="og", bufs=4))

        # PSUM: 8 banks.  acc (4) rotates through G/T/outT/final chunks;
        # s (2) per-head-pair score tiles; a (2) rowsums + A'T chunks.
        ps_acc = ctx.enter_context(tc.tile_pool(name="psacc", bufs=4, space="PSUM"))
        ps_s = ctx.enter_context(tc.tile_pool(name="pss", bufs=2, space="PSUM"))
        ps_a = ctx.enter_context(tc.tile_pool(name="psa", bufs=2, space="PSUM"))

        st = [{} for _ in range(B_PER_CORE)]

        def s_gram(b):
            """G[i*128:(i+1)*128, :] = sum_mc xt[mc]-slice^T @ xt[mc].
            All four g copies on DVE so consumers wait one proc."""
            xt_sb = st[b]["xt"]
            g_sb = gp.tile([128, 4, C], BF16, tag="g", name=f"g{b}")
            st[b]["g"] = g_sb
            pts = [
                ps_acc.tile([128, C], F32, tag="acc", name=f"pg{b}_{i}")
                for i in range(4)
            ]
            for mc in range(8):
                for i in range(4):
                    nc.tensor.matmul(
                        pts[i][:, :],
                        xt_sb[:, mc, i * 128 : (i + 1) * 128],
                        xt_sb[:, mc, :],
                        start=(mc == 0), stop=(mc == 7),
                    )
            for i in range(4):
                if i % 2 == 0:
                    nc.vector.tensor_copy(g_sb[:, i, :], pts[i][:, :])
                else:
                    nc.scalar.copy(g_sb[:, i, :], pts[i][:, :])

        def s_t(b):
            """T = G @ WkT; all four t copies on Act."""
            g_sb = st[b]["g"]
            wkt_sb = st[b]["wkt"]
            t_sb = tp.tile([128, 4, C], BF16, tag="t", name=f"t{b}")
            st[b]["t"] = t_sb
            pts = [
                ps_acc.tile([128, C], F32, tag="acc", name=f"pt{b}_{i}")
                for i in range(4)
            ]
            for j in range(4):
                for i in range(4):
                    nc.tensor.matmul(
                        pts[i][:, :],
                        g_sb[:, j, i * 128 : (i + 1) * 128],
                        wkt_sb[:, j, :],
                        start=(j == 0), stop=(j == 3),
                    )
            for i in range(4):
                if i % 2 == 0:
                    nc.vector.tensor_copy(t_sb[:, i, :], pts[i][:, :])
                else:
                    nc.scalar.copy(t_sb[:, i, :], pts[i][:, :])

        def s_scores(b):
            """scores per head-pair hp in its own PSUM tile; softmax over e
            (Act exp with free-axis accum, DVE recip, Act copy-scale); the
            normalized attn rows land in the block-diagonal slots of
            abd_pre, which the XBAR dma transposes into at_bd = attn^T."""
            wqt_sb = st[b]["wqt"]
            t_sb = st[b]["t"]
            es = smp.tile([128, 4, HD], BF16, tag="es", name=f"es{b}")
            rs = smp.tile([128, 4, 1], F32, tag="rs", name=f"rs{b}")
            rcp = smp.tile([128, 4, 1], F32, tag="rcp", name=f"rcp{b}")
            abd_pre = smp.tile([128, 4, 128], BF16, tag="abdp", name=f"abd_pre{b}")
            at_bd = smp.tile([128, 4, 128], BF16, tag="atbd", name=f"at_bd{b}")
            nc.vector.memset(abd_pre[:, :, :], 0.0)
            st[b]["at_bd"] = at_bd
            for hp in range(4):
                ps1 = ps_s.tile([128, 128], F32, tag="s1", name=f"ps1_{b}_{hp}")
                for j in range(4):
                    nc.tensor.matmul(
                        ps1[:, :],
                        wqt_sb[:, j, hp * 128 : (hp + 1) * 128],
                        t_sb[:, j, hp * 128 : (hp + 1) * 128],
                        start=(j == 0), stop=(j == 3),
                    )
                for hh in range(2):
                    psl = slice(hh * 64, hh * 64 + 64)
                    nc.scalar.activation(
                        es[psl, hp, :], ps1[psl, psl],
                        AF.Exp, scale=0.125,
                        accum_out=rs[psl, hp, :],
                    )
                nc.vector.reciprocal(rcp[:, hp, :], rs[:, hp, :])
                for hh in range(2):
                    psl = slice(hh * 64, hh * 64 + 64)
                    nc.scalar.activation(
                        abd_pre[psl, hp, psl], es[psl, hp, :],
                        AF.Copy, scale=rcp[psl, hp, :],
                    )
            # blockdiag(attn)^T == blockdiag(attn^T): transpose each pair
            # tile SBUF->SBUF on the DMA XBAR (off PE/PSUM entirely)
            for hp in range(4):
                nc.sync.dma_start(
                    at_bd[:, hp, :], abd_pre[:, hp, :], transpose=True
                )

        def s_apt(b):
            """A\'T[i-chunk, (h,d)] = wv-chunks^T @ at_bd (all copies DVE)."""
            wv_sb = st[b]["wv"]
            at_bd = st[b]["at_bd"]
            apt_sb = ap_.tile([128, 4, C], BF16, tag="apt", name=f"apt{b}")
            st[b]["apt"] = apt_sb
            for i in range(4):
                pa = ps_a.tile([128, C], F32, tag="a", name=f"pa{b}_{i}")
                for hp in range(4):
                    nc.tensor.matmul(
                        pa[:, hp * 128 : (hp + 1) * 128],
                        wv_sb[:, hp, i * 128 : (i + 1) * 128],
                        at_bd[:, hp, :],
                        start=True, stop=True,
                    )
                if i % 2 == 0:
                    nc.vector.tensor_copy(apt_sb[:, i, :], pa[:, :])
                else:
                    nc.scalar.copy(apt_sb[:, i, :], pa[:, :])

        def s_outt(b):
            """outT[mc-chunk, (h,d)] = sum_j xc[j, mc-slice]^T @ A'T[j].
            The psum->y copies (all Act) realize the transpose(2,3)
            scramble: y[(h%2)*64+a, h//2, (2mc+rr)*64+d] = outT[rr*64+a, (h,d)]."""
            xc_sb = st[b]["xc"]
            apt_sb = st[b]["apt"]
            y_sb = yp.tile([128, 4, N], BF16, tag="y", name=f"y{b}")
            st[b]["y"] = y_sb
            for mc in range(8):
                po = ps_acc.tile([128, 4, 128], F32, tag="acc", name=f"po{b}_{mc}")
                for j in range(4):
                    nc.tensor.matmul(
                        po[:, :, :],
                        xc_sb[:, j, mc * 128 : (mc + 1) * 128],
                        apt_sb[:, j, :],
                        start=(j == 0), stop=(j == 3),
                    )
                k = 0
                for rr in range(2):
                    for par in range(2):
                        dst = y_sb[
                            par * 64 : par * 64 + 64,
                            :,
                            (2 * mc + rr) * 64 : (2 * mc + rr) * 64 + 64,
                        ]
                        src_ = po[
                            rr * 64 : rr * 64 + 64, :, par * 64 : par * 64 + 64
                        ]
                        if (mc + k) % 2 == 0:
                            nc.vector.tensor_copy(dst, src_)
                        else:
                            nc.scalar.copy(dst, src_)
                        k += 1

        def s_final(b):
            """final[oc-chunk, :] = sum_j wot[j, oc-slice]^T @ y[j, :].
            j-outer / sh-inner shares each LDWEIGHTS across two matmuls.
            og is folded to 64 partitions: og64[p2, oc, k, s] =
            final[oc*128 + k*64 + p2, sh*512+s]; all og copies on DVE."""
            wot_sb = st[b]["wot"]
            y_sb = st[b]["y"]
            ogs = [
                ogp.tile([64, 4, 2, 512], BF16, tag="og", name=f"og{b}_{sh}")
                for sh in range(2)
            ]
            for oc in range(4):
                pf = [
                    ps_acc.tile([128, C], F32, tag="acc", name=f"pf{b}_{oc}_{sh}")
                    for sh in range(2)
                ]
                for j in range(4):
                    for sh in range(2):
                        nc.tensor.matmul(
                            pf[sh][:, :],
                            wot_sb[:, j, oc * 128 : (oc + 1) * 128],
                            y_sb[:, j, sh * 512 : (sh + 1) * 512],
                            start=(j == 0), stop=(j == 3),
                        )
                for sh in range(2):
                    for k in range(2):
                        if (oc + sh + k) % 2 == 0:
                            nc.vector.tensor_copy(
                                ogs[sh][:, oc, k, :],
                                pf[sh][k * 64 : (k + 1) * 64, :],
                            )
                        else:
                            nc.scalar.copy(
                                ogs[sh][:, oc, k, :],
                                pf[sh][k * 64 : (k + 1) * 64, :],
                            )
                if b == 1 and oc == 1:
                    nc.sync.dma_start(
                        out_d[1, 0, :, 0:2, :, :], ogs[0][:, 0:2, :, :]
                    )
                    nc.scalar.dma_start(
                        out_d[1, 1, :, 0:2, :, :], ogs[1][:, 0:2, :, :]
                    )
            if b == 0:
                nc.sync.dma_start(out_d[0, 0, :, :, :, :], ogs[0][:, :, :, :])
                nc.scalar.dma_start(out_d[0, 1, :, :, :, :], ogs[1][:, :, :, :])
            else:
                nc.sync.dma_start(out_d[1, 0, :, 2:4, :, :], ogs[0][:, 2:4, :, :])
                nc.scalar.dma_start(out_d[1, 1, :, 2:4, :, :], ogs[1][:, 2:4, :, :])

        # ---- loads ----
        xt0 = xtp.tile([128, 8, C], BF16, tag="xt", name="xt_sb0")
        xt1 = xtp.tile([128, 8, C], BF16, tag="xt", name="xt_sb1")
        xc0 = xcp.tile([128, 4, N], BF16, tag="xc", name="xc_sb0")
        xc1 = xcp.tile([128, 4, N], BF16, tag="xc", name="xc_sb1")
        st[0]["xt"], st[1]["xt"] = xt0, xt1
        st[0]["xc"], st[1]["xc"] = xc0, xc1

        nc.sync.dma_start(xt0[0:64, 0:4, :], xt_d[0, 0:64, 0:4, :])
        nc.scalar.dma_start(xt0[64:128, 0:4, :], xt_d[0, 64:128, 0:4, :])
        nc.sync.dma_start(xt0[0:64, 4:8, :], xt_d[0, 0:64, 4:8, :])
        nc.scalar.dma_start(xt0[64:128, 4:8, :], xt_d[0, 64:128, 4:8, :])

        w_sb = {}
        for wname in ("wkt", "wqt", "wv", "wot"):
            t = wp.tile([128, 4, C], BF16, tag=wname, name=f"w_{wname}")
            w_sb[wname] = t
            for b in range(B_PER_CORE):
                st[b][wname] = t
        nc.sync.dma_start(xt1[:, :, :], xt_d[1, :, :, :])
        nc.scalar.dma_start(w_sb["wkt"][:, :, :], wkt_d[:, :, :])
        nc.scalar.dma_start(w_sb["wqt"][:, :, :], wqt_d[:, :, :])
        nc.gpsimd.dma_start(xc0[:, :, :], xc_d[0, :, :, :])
        nc.gpsimd.dma_start(w_sb["wv"][:, :, :], wv_d[:, :, :])
        nc.gpsimd.dma_start(w_sb["wot"][:, :, :], wot_d[:, :, :])
        nc.gpsimd.dma_start(xc1[:, :, :], xc_d[1, :, :, :])

        # ---- schedule (a hint: the tile scheduler statically reorders
        # per-engine streams with its cost model) ----
        s_gram(0)
        s_t(0)
        s_scores(0)
        s_gram(1)
        s_t(1)
        s_scores(1)
        s_apt(0)
        s_apt(1)
        s_outt(0)
        s_outt(1)
        s_final(0)
        s_final(1)

    nc.compile()
    _split_excess_dma_waits(nc)
    if os.environ.get("K_MOVE_ACT_TABLE", "1") == "1":
        _postpone_act_table_load(nc)
    return nc


_PROGRAM = None


def _get_program():
    global _PROGRAM
    if _PROGRAM is None:
        _PROGRAM = build_program()
    return _PROGRAM


def make_in_maps(x, Wq, Wk, Wv, Wo):
    import ml_dtypes

    bf = ml_dtypes.bfloat16
    # permute spatial axis n = 16a + r -> m = 64r + a
    xm = (
        x.reshape(16, C, 64, 16)
        .transpose(0, 1, 3, 2)
        .reshape(16, C, N)
    )
    # xc: [b, 128, cc, m]  (X, channel-partition)
    xc = np.ascontiguousarray(
        xm.reshape(16, 4, 128, N).transpose(0, 2, 1, 3).astype(bf)
    )
    # xt: [b, 128, mc, c]  (X^T, m-partition)
    xt = np.ascontiguousarray(
        xm.transpose(0, 2, 1).reshape(16, 8, 128, C).transpose(0, 2, 1, 3).astype(bf)
    )

    def _wt(W):
        return np.ascontiguousarray(
            W.T.reshape(4, 128, C).transpose(1, 0, 2).astype(bf)
        )

    wkt, wqt, wot = _wt(Wk), _wt(Wq), _wt(Wo)
    # wv: [128 (hh,e), hp, c]: row (2*hp+hh)*64+e of Wv
    wv = np.ascontiguousarray(
        Wv.reshape(4, 2, HD, C).transpose(1, 2, 0, 3).reshape(128, 4, C).astype(bf)
    )
    in_maps = []
    for c in range(N_CORES):
        bsl = slice(c * B_PER_CORE, (c + 1) * B_PER_CORE)
        in_maps.append(
            {
                "xt": np.ascontiguousarray(xt[bsl]),
                "xc": np.ascontiguousarray(xc[bsl]),
                "wkt": wkt,
                "wqt": wqt,
                "wv": wv,
                "wot": wot,
            }
        )
    return in_maps


def kernel(x, Wq, bq, Wk, bk, Wv, bv, Wo, bo, _trace=False):
    # biases are zeros by construction in this problem (spec fill="zeros");
    # they are not applied on-device.
    nc = _get_program()
    in_maps = make_in_maps(x, Wq, Wk, Wv, Wo)
    res = run_bass_kernel_spmd(nc, in_maps, list(range(N_CORES)), trace=_trace)
    outs = [
        np.asarray(res.results[c]["out"]).astype(np.float32) for c in range(N_CORES)
    ]
    # out [b, sh, p2, oc, k, s] -> F[b, oc*128 + k*64 + p2, sh*512 + s];
    # the spatial index is already the true flat index (no inverse perm)
    full = (
        np.concatenate(outs, axis=0)
        .transpose(0, 3, 4, 2, 1, 5)
        .reshape(16, C, N)
        .reshape(16, C, 32, 32)
    )
    if _trace:
        return full, res
    return full
